# revision 36
# baseline (speedup 1.0000x reference)
"""Trainium2 Bass kernel for nn_MultiHeadAttention_88923002896848.

MHA with KV-cache concat: out = MHA(query; [cache;key_in]; [cache;value_in]).
Shapes: B=128, T1=188, LC=70, T2=258, F=512, H=8, DK=64. fp32 I/O.

Strategy (8 NeuronCores, data-parallel over batch, 16 batches/core):
  - Host: activations to feature-major [b, F, T] layouts; weights [fin,fout].
  - All matmuls fp16. Attention transposed (S^T = k-chunks x q) so exp output
    E^T feeds PV with zero on-chip transposes; exp is max-free (logits ~N(0,1))
    with a constant -3 shift; softmax denominator comes free from a ones-column
    appended to time-major V (row 64 of the PV psum).
  - Normalization runs entirely off the PE: Vector reciprocal of the
    denominator row, GpSimd partition_broadcast across the 64 head dims,
    Vector multiply into the (feature-major) context buffer.
  - Output projection is pair-folded feature-major (N=376, M=128 always);
    the kernel emits outT [b, F, T1] fp16 and the host transposes/upcasts.
  - Software pipelining across batch pairs: while pair p's attention runs,
    the PE also executes Q/K/V projections of pair p+1 and the output
    projection of pair p-1 as filler groups slotted between attention
    groups, keeping the PE dense so the HAM clock gate stays at 2.4 GHz.
"""

import numpy as np

NCORES = 8
B, T1, LC, F, H = 128, 188, 70, 512, 8
DK = F // H            # 64
T2 = LC + T1           # 258
P = 128
KO = F // P            # 4 fin/fout tiles of 128
NB = B // NCORES       # 16 batches per core
NPAIR = NB // 2
SCALE = 1.0 / np.sqrt(DK)
EXP_SHIFT = -3.0       # exp(scale*s + shift); cancels in the softmax ratio

# T2 chunks aligned to the cache/key seam: (size, (source, t0, t1))
T2_CHUNKS = [(LC, ("cache", 0, LC)), (128, ("key", 0, 128)), (T1 - 128, ("key", 128, T1))]

_BUILT = None


def _build():
    import concourse.bacc as bacc
    import concourse.mybir as mybir
    import concourse.tile as tile
    from concourse.bass import ts
    from contextlib import ExitStack

    dt = mybir.dt
    f32, f16 = dt.float32, dt.float16
    AF = mybir.ActivationFunctionType

    nc = bacc.Bacc(trn_type="TRN2")

    qT = nc.dram_tensor("qT", [NB, F, T1], f16, kind="ExternalInput")
    keyT = nc.dram_tensor("keyT", [NB, F, T1], f16, kind="ExternalInput")
    valT = nc.dram_tensor("valT", [NB, F, T1], f16, kind="ExternalInput")
    cachT = nc.dram_tensor("cachT", [F, NB, LC], f16, kind="ExternalInput")
    wq_d = nc.dram_tensor("wq", [F, F], f16, kind="ExternalInput")
    wk_d = nc.dram_tensor("wk", [F, F], f16, kind="ExternalInput")
    wv_d = nc.dram_tensor("wv", [F, F], f16, kind="ExternalInput")
    wo_d = nc.dram_tensor("wo", [F, F], f16, kind="ExternalInput")
    out_d = nc.dram_tensor("out", [NB, F, T1], f16, kind="ExternalOutput")

    with tile.TileContext(nc) as tc, ExitStack() as ctx:
        consts = ctx.enter_context(tc.tile_pool(name="consts", bufs=1))
        iobuf = ctx.enter_context(tc.tile_pool(name="iobuf", bufs=2))
        act16 = ctx.enter_context(tc.tile_pool(name="act16", bufs=2))
        small = ctx.enter_context(tc.tile_pool(name="small", bufs=2))
        pproj = ctx.enter_context(tc.tile_pool(name="pproj", bufs=2, space="PSUM"))
        pscore = ctx.enter_context(tc.tile_pool(name="pscore", bufs=3, space="PSUM"))
        pctx = ctx.enter_context(tc.tile_pool(name="pctx", bufs=3, space="PSUM"))

        # ---- constants ----
        # DMA order: wk + cache first so the KTc preamble matmuls can start
        # as early as possible, overlapping the remaining input streams
        w_sb = {}
        for nm, drt in (("wk", wk_d), ("wq", wq_d), ("wv", wv_d), ("wo", wo_d)):
            wt = consts.tile([P, KO, F], f16, name=f"{nm}_sb", tag=f"{nm}_sb")
            w_sb[nm] = wt
        cache_all = consts.tile([P, KO, NB, LC], f16, name="cache_all")
        nc.sync.dma_start(w_sb["wk"][:], wk_d.rearrange("(o p) f -> p o f", p=P))
        nc.sync.dma_start(
            cache_all.rearrange("p o b t -> p o (b t)"),
            cachT.rearrange("(o p) b t -> p o (b t)", p=P),
        )
        for nm, drt in (("wq", wq_d), ("wv", wv_d), ("wo", wo_d)):
            nc.sync.dma_start(w_sb[nm][:], drt.rearrange("(o p) f -> p o f", p=P))
        biasm3 = consts.tile([P, 1], f32, name="biasm3")
        nc.vector.memset(biasm3[:], EXP_SHIFT)
        ones_col = consts.tile([1, DK], f32, name="ones_col")
        nc.vector.memset(ones_col[:], 1.0)

        hb_row = consts.tile([1, P], f32, name="hb_row")
        nc.vector.memset(hb_row[:], 1.0)

        # ---- K projection of all cache frames (feature-major, fp16) ----
        KTc = consts.tile([P, KO, NB, LC], f16, name="KTc")
        NTOT = NB * LC  # 1120
        cch = [(0, 374), (374, 374), (748, NTOT - 748)]
        for fo in range(KO):
            for c0, cn in cch:
                pkc = pproj.tile([P, F], f32, tag="proj", name="pkc")
                for k in range(KO):
                    nc.tensor.matmul(
                        pkc[:, :cn],
                        w_sb["wk"][:, k, ts(fo, P)],
                        cache_all[:, k].rearrange("p b t -> p (b t)")[:, c0 : c0 + cn],
                        start=(k == 0),
                        stop=(k == KO - 1),
                    )
                nc.scalar.copy(
                    KTc[:, fo].rearrange("p b t -> p (b t)")[:, c0 : c0 + cn],
                    pkc[:, :cn],
                )

        # ================= pipelined stages =================

        def dma_pair(pr):
            qp = iobuf.tile([P, KO, 2, T1], f16, tag="qp", name="qp")
            kp = iobuf.tile([P, KO, 2, T1], f16, tag="kp", name="kp")
            vp = iobuf.tile([P, KO, 2, T1], f16, tag="vp", name="vp")
            for lb in range(2):
                b = 2 * pr + lb
                nc.sync.dma_start(qp[:, :, lb], qT[b].rearrange("(o p) t -> p o t", p=P))
                nc.sync.dma_start(kp[:, :, lb], keyT[b].rearrange("(o p) t -> p o t", p=P))
                nc.sync.dma_start(vp[:, :, lb], valT[b].rearrange("(o p) t -> p o t", p=P))
            return qp, kp, vp

        def qk_proj_group(io, w, dst, fo):
            # one fo block of the pair-folded Q or K projection (N=376)
            pq = pproj.tile([P, F], f32, tag="proj", name="pq")
            for k in range(KO):
                nc.tensor.matmul(
                    pq[:, : 2 * T1],
                    w_sb[w][:, k, ts(fo, P)],
                    io[:, k].rearrange("p b t -> p (b t)"),
                    start=(k == 0),
                    stop=(k == KO - 1),
                )
            if w == "wq":
                nc.scalar.copy(dst[:, fo].rearrange("p b t -> p (b t)"), pq[:, : 2 * T1])
            else:
                nc.vector.tensor_copy(
                    dst[:, fo].rearrange("p b t -> p (b t)"), pq[:, : 2 * T1]
                )

        def v_proj_group(vp, pr, lb, ci, vts):
            # one T2 chunk of batch (pr, lb)'s V projection, time-major + ones col
            tcn, (src, s0, s1) = T2_CHUNKS[ci]
            b = 2 * pr + lb
            pv = pproj.tile([P, F], f32, tag="proj", name="pv")[:tcn]
            for k in range(KO):
                if src == "cache":
                    lhsT = cache_all[:, k, b, :]
                else:
                    lhsT = vp[:, k, lb, s0:s1]
                nc.tensor.matmul(
                    pv[:, :],
                    lhsT,
                    w_sb["wv"][:, k],
                    start=(k == 0),
                    stop=(k == KO - 1),
                )
            vt = vts[lb][ci]
            nc.vector.tensor_copy(vt[:, :, 0:DK], pv.rearrange("t (h d) -> t h d", d=DK))
            nc.gpsimd.memset(vt[:, :, DK : DK + 1], 1.0)

        def o_proj_group(ctxs, pr, fo, lbs=(0, 1)):
            # one fo block of the pair-folded output projection (N=376),
            # feature-major output; lbs selects batches (epilogue splits them)
            nlb = len(lbs)
            po = pproj.tile([P, F], f32, tag="proj", name="po")
            for k in range(KO):
                if nlb == 2:
                    rhs = ctxs[:, k].rearrange("p b t -> p (b t)")
                else:
                    rhs = ctxs[:, k, lbs[0], :]
                nc.tensor.matmul(
                    po[:, : nlb * T1],
                    w_sb["wo"][:, k, ts(fo, P)],
                    rhs,
                    start=(k == 0),
                    stop=(k == KO - 1),
                )
            ob = small.tile([P, 2, T1], f16, tag="ob", name="ob", bufs=3)
            nc.scalar.copy(
                ob[:, : nlb].rearrange("p b t -> p (b t)"), po[:, : nlb * T1]
            )
            for i, lb in enumerate(lbs):
                b = 2 * pr + lb
                nc.sync.dma_start(out_d[b, ts(fo, P), :], ob[:, i, :])

        # ---- prologue: pair 0 inputs + projections ----
        qp0, kp0, vp0 = dma_pair(0)
        q16 = act16.tile([P, KO, 2, T1], f16, tag="q16", name="q16")
        k16 = act16.tile([P, KO, 2, T1], f16, tag="k16", name="k16")
        for fo in range(KO):
            qk_proj_group(qp0, "wq", q16, fo)
        for fo in range(KO):
            qk_proj_group(kp0, "wk", k16, fo)
        vts = [
            [
                act16.tile([P, H, DK + 1], f16, tag=f"vt{lb}{ci}", name=f"vt{lb}{ci}", bufs=2)[:tcn]
                for ci, (tcn, _) in enumerate(T2_CHUNKS)
            ]
            for lb in range(2)
        ]
        for lb in range(2):
            for ci in range(len(T2_CHUNKS)):
                v_proj_group(vp0, 0, lb, ci, vts)

        prev = None  # (pr, ctxs) of previous pair, for the deferred O projection
        vts_deferred = None  # (vp, vts) of the last pair, projected in-pair

        for pr in range(NPAIR):
            # queue next pair's DMA + build its filler group list.  The last
            # pair's V projection is deferred into the last pair itself (VP
            # fillers first there) so the tail keeps the PE dense.
            fillers = []
            if pr == NPAIR - 1 and vts_deferred is not None:
                vpn_d, vtsn_d = vts_deferred
                for lb in range(2):
                    for ci in range(len(T2_CHUNKS)):
                        fillers.append(
                            lambda lb=lb, ci=ci: v_proj_group(vpn_d, pr, lb, ci, vtsn_d)
                        )
                vts = vtsn_d
            if pr + 1 < NPAIR:
                qpn, kpn, vpn = dma_pair(pr + 1)
                q16n = act16.tile([P, KO, 2, T1], f16, tag="q16", name="q16n")
                k16n = act16.tile([P, KO, 2, T1], f16, tag="k16", name="k16n")
                vtsn = [
                    [
                        act16.tile(
                            [P, H, DK + 1], f16, tag=f"vt{lb}{ci}", name=f"vtn{lb}{ci}", bufs=2
                        )[:tcn]
                        for ci, (tcn, _) in enumerate(T2_CHUNKS)
                    ]
                    for lb in range(2)
                ]
                for fo in range(KO):
                    fillers.append(lambda fo=fo: qk_proj_group(qpn, "wq", q16n, fo))
                for fo in range(KO):
                    fillers.append(lambda fo=fo: qk_proj_group(kpn, "wk", k16n, fo))
                if pr + 1 == NPAIR - 1:
                    vts_deferred = (vpn, vtsn)
                    vtsn = None
                else:
                    for lb in range(2):
                        for ci in range(len(T2_CHUNKS)):
                            fillers.append(
                                lambda lb=lb, ci=ci: v_proj_group(vpn, pr + 1, lb, ci, vtsn)
                            )
            else:
                q16n = k16n = vtsn = None
            if prev is not None:
                ppr, pctxs = prev
                for fo in range(KO):
                    fillers.append(lambda fo=fo: o_proj_group(pctxs, ppr, fo))
            fillers.reverse()  # pop() from the front

            def filler(n=1):
                for _ in range(n):
                    if fillers:
                        fillers.pop()()

            q16c, k16c, vtsc = q16, k16, vts

            # E tiles for the pair: [tcn, h, lb, t] fp16
            E = [
                act16.tile([P, H, 2, T1], f16, tag=f"E{ci}", name=f"E{ci}", bufs=2)[:tcn]
                for ci, (tcn, _) in enumerate(T2_CHUNKS)
            ]
            # final pair keeps cu in f32 so its normalization can use the
            # PE-matmul broadcast (uniform f32) without the f16 cast step
            cudt = f32 if pr == NPAIR - 1 else f16
            cu = [
                small.tile(
                    [DK + 1, H, T1], cudt,
                    tag=f"cu{lb}{'F' if pr == NPAIR - 1 else ''}",
                    name=f"cu{lb}", bufs=1 if pr == NPAIR - 1 else 2,
                )
                for lb in range(2)
            ]
            ctxs = small.tile([P, KO, 2, T1], f16, tag="ctxs", name="ctxs", bufs=2)

            def scores_fo(fo):
                # S^T + exp for head pair fo, both batches, all chunks
                for ci, (tcn, (src, s0, s1)) in enumerate(T2_CHUNKS):
                    for j in range(2):
                        pss = pscore.tile([P, 2, 256], f32, tag="pss", name="pss")[:tcn]
                        for lb in range(2):
                            if src == "cache":
                                lhsT = KTc[ts(j, DK), fo, 2 * pr + lb, :]
                            else:
                                lhsT = k16c[ts(j, DK), fo, lb, s0:s1]
                            # one accumulation group for both batches: the lb=1
                            # write lands on cleared has_written bits, so it
                            # overwrites its own column range (no second
                            # whole-bank clear, no intermediate drain tail)
                            nc.tensor.matmul(
                                pss[:, lb, :T1],
                                lhsT,
                                q16c[ts(j, DK), fo, lb, :],
                                start=(lb == 0),
                                stop=(lb == 1),
                            )
                        nc.scalar.activation(
                            E[ci][:, 2 * fo + j, :, :],
                            pss[:, :, :T1],
                            AF.Exp,
                            bias=biasm3[:tcn, :],
                            scale=SCALE,
                        )

            def pv_fo(fo, lbs=(0, 1)):
                # PV with fused denominator row; drain unnormalized to cu (fp16)
                for lb in lbs:
                    pc = pctx.tile([DK + 1, 2, 256], f32, tag="pc", name="pc")
                    for j in range(2):
                        h = 2 * fo + j
                        for ci, (tcn, _) in enumerate(T2_CHUNKS):
                            # both heads share one accumulation group (j=1's
                            # first write overwrites on cleared bits) to avoid
                            # a second bank-clear and mid-group drain tail
                            nc.tensor.matmul(
                                pc[:, j, :T1],
                                vtsc[lb][ci][:, h, :],
                                E[ci][:, h, lb, :],
                                start=(j == 0 and ci == 0),
                                stop=(j == 1 and ci == len(T2_CHUNKS) - 1),
                            )
                    nc.vector.tensor_copy(cu[lb][:, 2 * fo : 2 * fo + 2, :], pc[:, :, :T1])

            # ---- attention sequence with interleaved filler groups ----
            if pr < NPAIR - 1:
                scores_fo(0)
                filler(2)
                scores_fo(1)
                filler(2)
                pv_fo(0)
                filler(2)
                scores_fo(2)
                filler(2)
                pv_fo(1)
                filler(2)
                scores_fo(3)
                filler(2)
                pv_fo(2)
                filler(2)
                pv_fo(3)
                while fillers:
                    filler(1)

            # ---- normalization (no PE): recip + partition broadcast + mul ----
            def norm_batch(lb):
                dj = small.tile([1, 2, KO, T1], f32, tag="dj", name="dj", bufs=2)
                for j in range(2):
                    nc.vector.tensor_copy(
                        dj[:, j, :, :],
                        cu[lb][DK : DK + 1, :, :].rearrange("p (f j) t -> p j f t", j=2)[
                            :, j, :, :
                        ],
                    )
                rjf = small.tile([1, 2, KO, T1], f32, tag="rjf", name="rjf", bufs=2)
                nc.vector.reciprocal_approx_fast(
                    out=rjf.rearrange("p j f t -> p (j f t)"),
                    in_=dj.rearrange("p j f t -> p (j f t)"),
                )
                rj = small.tile([1, 2, KO, T1], f16, tag="rj", name="rj", bufs=2)
                nc.vector.tensor_copy(
                    rj.rearrange("p j f t -> p (j f t)"),
                    rjf.rearrange("p j f t -> p (j f t)"),
                )
                for j in range(2):
                    rb = small.tile([DK, KO, T1], f16, tag=f"rb{j}", name=f"rb{j}", bufs=2)
                    nc.gpsimd.partition_broadcast(rb[:], rj[:, j, :, :])
                    nc.vector.tensor_mul(
                        ctxs[ts(j, DK), :, lb, :],
                        cu[lb][0:DK, :, :].rearrange("p (f j) t -> p j f t", j=2)[:, j],
                        rb[:],
                    )

            def heartbeat(n):
                # dependency-free matmuls that keep the PE activity monitor
                # from re-throttling the clock across short dependency stalls
                # in the pipeline tail (scores are done, so pss tiles are free)
                for _ in range(n):
                    hp = pscore.tile([P, 2, 256], f32, tag="pss", name="hp")
                    nc.tensor.matmul(
                        hp[:DK, 0, :P], ones_col[:], hb_row[:], start=True, stop=True
                    )

            def norm_batch_final(lb):
                # tail variant: PE is otherwise idle here, so broadcast the
                # reciprocal across partitions with a K=1 ones-matmul (fast)
                # instead of the gpsimd partition_broadcast (1.3us each)
                dj = small.tile([1, 2, KO, T1], f32, tag="djF", name="djF", bufs=1)
                for j in range(2):
                    nc.vector.tensor_copy(
                        dj[:, j, :, :],
                        cu[lb][DK : DK + 1, :, :].rearrange("p (f j) t -> p j f t", j=2)[
                            :, j, :, :
                        ],
                    )
                rjf = small.tile([1, 2, KO, T1], f32, tag="rjfF", name="rjfF", bufs=1)
                for j in range(2):
                    nc.vector.reciprocal_approx_fast(
                        out=rjf[:, j].rearrange("p f t -> p (f t)"),
                        in_=dj[:, j].rearrange("p f t -> p (f t)"),
                    )
                for j in range(2):
                    for q in range(2):
                        pbt = pscore.tile([P, 2, 256], f32, tag="pss", name="pbt")
                        nc.tensor.matmul(
                            pbt[:DK, :, :T1],
                            ones_col[:],
                            rjf[:, j, 2 * q : 2 * q + 2, :],
                            start=True,
                            stop=True,
                        )
                        nc.vector.tensor_mul(
                            ctxs[ts(j, DK), 2 * q : 2 * q + 2, lb, :],
                            cu[lb][0:DK, :, :].rearrange("p (f j) t -> p j f t", j=2)[
                                :, j, 2 * q : 2 * q + 2, :
                            ],
                            pbt[:DK, :, :T1],
                        )

            if pr == NPAIR - 1:
                # final pair: drain batch 0's attention first so its
                # normalization and output projection overlap batch 1's PV,
                # keeping the PE busy through the pipeline tail
                scores_fo(0)
                filler(3)
                scores_fo(1)
                filler(3)
                pv_fo(0, lbs=(0,))
                scores_fo(2)
                filler(2)
                pv_fo(1, lbs=(0,))
                scores_fo(3)
                filler(2)
                pv_fo(2, lbs=(0,))
                filler(1)
                pv_fo(3, lbs=(0,))
                while fillers:
                    filler(1)
                pv_fo(0, lbs=(1,))
                pv_fo(1, lbs=(1,))
                pv_fo(2, lbs=(1,))
                pv_fo(3, lbs=(1,))
                norm_batch_final(0)
                for fo in range(KO):
                    o_proj_group(ctxs, pr, fo, lbs=(0,))
                norm_batch_final(1)
                for fo in range(KO):
                    o_proj_group(ctxs, pr, fo, lbs=(1,))
            else:
                for lb in range(2):
                    norm_batch(lb)

            prev = (pr, ctxs)
            q16, k16, vts = q16n, k16n, vtsn

    nc.compile()
    return nc


def _get_built():
    global _BUILT
    if _BUILT is None:
        _BUILT = _build()
    return _BUILT


def _numpy_ref(query, key_in, value_in, cache, mask, Wq, bq, Wk, bk, Wv, bv, Wo, bo):
    # Fallback oracle (only used if mask/bias assumptions are violated).
    k_full = np.concatenate([cache, key_in], axis=1)
    v_full = np.concatenate([cache, value_in], axis=1)

    def proj(x, W, b):
        y = x @ W.T + b
        return y.reshape(x.shape[0], x.shape[1], H, DK).transpose(0, 2, 1, 3)

    q = proj(query, Wq, bq)
    k = proj(k_full, Wk, bk)
    v = proj(v_full, Wv, bv)
    s = np.einsum("bhqd,bhkd->bhqk", q, k) / np.sqrt(np.float32(DK))
    m = mask[:, None, :, :]
    s = np.where(m, s, -10000.0)
    s = s - s.max(-1, keepdims=True)
    e = np.exp(s)
    a = e / e.sum(-1, keepdims=True)
    a = np.where(m, a, 0.0)
    ctx = np.einsum("bhqk,bhkd->bhqd", a, v)
    ctx = ctx.transpose(0, 2, 1, 3).reshape(query.shape[0], query.shape[1], F)
    return (ctx @ Wo.T + bo).astype(np.float32)


def kernel(**inputs):
    q = np.asarray(inputs["query"], np.float32)
    key_in = np.asarray(inputs["key_in"], np.float32)
    value_in = np.asarray(inputs["value_in"], np.float32)
    cache = np.asarray(inputs["cache"], np.float32)
    mask = np.asarray(inputs["mask"])
    Wq = np.asarray(inputs["Wq"], np.float32)
    Wk = np.asarray(inputs["Wk"], np.float32)
    Wv = np.asarray(inputs["Wv"], np.float32)
    Wo = np.asarray(inputs["Wo"], np.float32)
    bq = np.asarray(inputs["bq"], np.float32)
    bk = np.asarray(inputs["bk"], np.float32)
    bv = np.asarray(inputs["bv"], np.float32)
    bo = np.asarray(inputs["bo"], np.float32)

    if (not mask.all()) or any(np.any(b != 0) for b in (bq, bk, bv, bo)):
        return _numpy_ref(q, key_in, value_in, cache, mask, Wq, bq, Wk, bk, Wv, bv, Wo, bo)

    nc = _get_built()

    wq_t = np.ascontiguousarray(Wq.T).astype(np.float16)
    wk_t = np.ascontiguousarray(Wk.T).astype(np.float16)
    wv_t = np.ascontiguousarray(Wv.T).astype(np.float16)
    wo_t = np.ascontiguousarray(Wo.T).astype(np.float16)

    in_maps = []
    for c in range(NCORES):
        sl = slice(c * NB, (c + 1) * NB)
        in_maps.append(
            {
                "qT": np.ascontiguousarray(q[sl].transpose(0, 2, 1)).astype(np.float16),
                "keyT": np.ascontiguousarray(key_in[sl].transpose(0, 2, 1)).astype(np.float16),
                "valT": np.ascontiguousarray(value_in[sl].transpose(0, 2, 1)).astype(np.float16),
                "cachT": np.ascontiguousarray(cache[sl].transpose(2, 0, 1)).astype(np.float16),
                "wq": wq_t,
                "wk": wk_t,
                "wv": wv_t,
                "wo": wo_t,
            }
        )

    from concourse.bass_utils import run_bass_kernel_spmd

    res = run_bass_kernel_spmd(nc, in_maps, core_ids=list(range(NCORES)))
    kernel._last_results = res
    return np.concatenate(
        [np.ascontiguousarray(r["out"].transpose(0, 2, 1)).astype(np.float32) for r in res.results],
        axis=0,
    )


# revision 37
# speedup vs baseline: 1.0389x; 1.0389x over previous
"""Trainium2 Bass kernel for nn_MultiHeadAttention_88923002896848.

MHA with KV-cache concat: out = MHA(query; [cache;key_in]; [cache;value_in]).
Shapes: B=128, T1=188, LC=70, T2=258, F=512, H=8, DK=64. fp32 I/O.

Strategy (8 NeuronCores, data-parallel over batch, 16 batches/core):
  - Host: activations to feature-major [b, F, T] layouts; weights [fin,fout].
  - All matmuls fp16. Attention transposed (S^T = k-chunks x q) so exp output
    E^T feeds PV with zero on-chip transposes; exp is max-free (logits ~N(0,1))
    with a constant -3 shift; softmax denominator comes free from a ones-column
    appended to time-major V (row 64 of the PV psum).
  - Normalization runs entirely off the PE: Vector reciprocal of the
    denominator row, GpSimd partition_broadcast across the 64 head dims,
    Vector multiply into the (feature-major) context buffer.
  - Output projection is pair-folded feature-major (N=376, M=128 always);
    the kernel emits outT [b, F, T1] fp16 and the host transposes/upcasts.
  - Software pipelining across batch pairs: while pair p's attention runs,
    the PE also executes Q/K/V projections of pair p+1 and the output
    projection of pair p-1 as filler groups slotted between attention
    groups, keeping the PE dense so the HAM clock gate stays at 2.4 GHz.
"""

import numpy as np

NCORES = 8
B, T1, LC, F, H = 128, 188, 70, 512, 8
DK = F // H            # 64
T2 = LC + T1           # 258
P = 128
KO = F // P            # 4 fin/fout tiles of 128
NB = B // NCORES       # 16 batches per core
NPAIR = NB // 2
SCALE = 1.0 / np.sqrt(DK)
EXP_SHIFT = -3.0       # exp(scale*s + shift); cancels in the softmax ratio

# T2 chunks aligned to the cache/key seam: (size, (source, t0, t1))
T2_CHUNKS = [(LC, ("cache", 0, LC)), (128, ("key", 0, 128)), (T1 - 128, ("key", 128, T1))]

_BUILT = None


def _build():
    import concourse.bacc as bacc
    import concourse.mybir as mybir
    import concourse.tile as tile
    from concourse.bass import ts
    from contextlib import ExitStack

    dt = mybir.dt
    f32, f16 = dt.float32, dt.float16
    AF = mybir.ActivationFunctionType

    nc = bacc.Bacc(trn_type="TRN2")

    qT = nc.dram_tensor("qT", [NB, F, T1], f16, kind="ExternalInput")
    keyT = nc.dram_tensor("keyT", [NB, F, T1], f16, kind="ExternalInput")
    valT = nc.dram_tensor("valT", [NB, F, T1], f16, kind="ExternalInput")
    cachT = nc.dram_tensor("cachT", [F, NB, LC], f16, kind="ExternalInput")
    wq_d = nc.dram_tensor("wq", [F, F], f16, kind="ExternalInput")
    wk_d = nc.dram_tensor("wk", [F, F], f16, kind="ExternalInput")
    wv_d = nc.dram_tensor("wv", [F, F], f16, kind="ExternalInput")
    wo_d = nc.dram_tensor("wo", [F, F], f16, kind="ExternalInput")
    out_d = nc.dram_tensor("out", [NB, F, T1], f16, kind="ExternalOutput")

    with tile.TileContext(nc) as tc, ExitStack() as ctx:
        consts = ctx.enter_context(tc.tile_pool(name="consts", bufs=1))
        iobuf = ctx.enter_context(tc.tile_pool(name="iobuf", bufs=2))
        act16 = ctx.enter_context(tc.tile_pool(name="act16", bufs=2))
        small = ctx.enter_context(tc.tile_pool(name="small", bufs=2))
        pproj = ctx.enter_context(tc.tile_pool(name="pproj", bufs=2, space="PSUM"))
        pscore = ctx.enter_context(tc.tile_pool(name="pscore", bufs=4, space="PSUM"))
        pctx = ctx.enter_context(tc.tile_pool(name="pctx", bufs=2, space="PSUM"))

        # ---- constants ----
        # DMA order: wk + cache first so the KTc preamble matmuls can start
        # as early as possible, overlapping the remaining input streams
        w_sb = {}
        for nm, drt in (("wk", wk_d), ("wq", wq_d), ("wv", wv_d), ("wo", wo_d)):
            wt = consts.tile([P, KO, F], f16, name=f"{nm}_sb", tag=f"{nm}_sb")
            w_sb[nm] = wt
        cache_all = consts.tile([P, KO, NB, LC], f16, name="cache_all")
        nc.sync.dma_start(w_sb["wk"][:], wk_d.rearrange("(o p) f -> p o f", p=P))
        nc.sync.dma_start(
            cache_all.rearrange("p o b t -> p o (b t)"),
            cachT.rearrange("(o p) b t -> p o (b t)", p=P),
        )
        for nm, drt in (("wq", wq_d), ("wv", wv_d), ("wo", wo_d)):
            nc.sync.dma_start(w_sb[nm][:], drt.rearrange("(o p) f -> p o f", p=P))
        biasm3 = consts.tile([P, 1], f32, name="biasm3")
        nc.vector.memset(biasm3[:], EXP_SHIFT)
        ones_col = consts.tile([1, DK], f32, name="ones_col")
        nc.vector.memset(ones_col[:], 1.0)

        hb_row = consts.tile([1, P], f32, name="hb_row")
        nc.vector.memset(hb_row[:], 1.0)

        # ---- K projection of all cache frames (feature-major, fp16) ----
        KTc = consts.tile([P, KO, NB, LC], f16, name="KTc")
        NTOT = NB * LC  # 1120
        cch = [(0, 374), (374, 374), (748, NTOT - 748)]
        for fo in range(KO):
            for c0, cn in cch:
                pkc = pproj.tile([P, F], f32, tag="proj", name="pkc")
                for k in range(KO):
                    nc.tensor.matmul(
                        pkc[:, :cn],
                        w_sb["wk"][:, k, ts(fo, P)],
                        cache_all[:, k].rearrange("p b t -> p (b t)")[:, c0 : c0 + cn],
                        start=(k == 0),
                        stop=(k == KO - 1),
                    )
                nc.scalar.copy(
                    KTc[:, fo].rearrange("p b t -> p (b t)")[:, c0 : c0 + cn],
                    pkc[:, :cn],
                )

        # ================= pipelined stages =================

        def dma_pair(pr):
            qp = iobuf.tile([P, KO, 2, T1], f16, tag="qp", name="qp")
            kp = iobuf.tile([P, KO, 2, T1], f16, tag="kp", name="kp")
            vp = iobuf.tile([P, KO, 2, T1], f16, tag="vp", name="vp")
            for lb in range(2):
                b = 2 * pr + lb
                nc.sync.dma_start(qp[:, :, lb], qT[b].rearrange("(o p) t -> p o t", p=P))
                nc.sync.dma_start(kp[:, :, lb], keyT[b].rearrange("(o p) t -> p o t", p=P))
                nc.sync.dma_start(vp[:, :, lb], valT[b].rearrange("(o p) t -> p o t", p=P))
            return qp, kp, vp

        def qk_proj_group(io, w, dst, fo):
            # one fo block of the pair-folded Q or K projection (N=376)
            pq = pproj.tile([P, F], f32, tag="proj", name="pq")
            for k in range(KO):
                nc.tensor.matmul(
                    pq[:, : 2 * T1],
                    w_sb[w][:, k, ts(fo, P)],
                    io[:, k].rearrange("p b t -> p (b t)"),
                    start=(k == 0),
                    stop=(k == KO - 1),
                )
            if w == "wq":
                nc.scalar.copy(dst[:, fo].rearrange("p b t -> p (b t)"), pq[:, : 2 * T1])
            else:
                nc.vector.tensor_copy(
                    dst[:, fo].rearrange("p b t -> p (b t)"), pq[:, : 2 * T1]
                )

        def v_proj_group(vp, pr, lb, ci, vts):
            # one T2 chunk of batch (pr, lb)'s V projection, time-major + ones col
            tcn, (src, s0, s1) = T2_CHUNKS[ci]
            b = 2 * pr + lb
            pv = pproj.tile([P, F], f32, tag="proj", name="pv")[:tcn]
            for k in range(KO):
                if src == "cache":
                    lhsT = cache_all[:, k, b, :]
                else:
                    lhsT = vp[:, k, lb, s0:s1]
                nc.tensor.matmul(
                    pv[:, :],
                    lhsT,
                    w_sb["wv"][:, k],
                    start=(k == 0),
                    stop=(k == KO - 1),
                )
            vt = vts[lb][ci]
            nc.vector.tensor_copy(vt[:, :, 0:DK], pv.rearrange("t (h d) -> t h d", d=DK))
            nc.gpsimd.memset(vt[:, :, DK : DK + 1], 1.0)

        def o_proj_group(ctxs, pr, fo, lbs=(0, 1)):
            # one fo block of the pair-folded output projection (N=376),
            # feature-major output; lbs selects batches (epilogue splits them)
            nlb = len(lbs)
            po = pproj.tile([P, F], f32, tag="proj", name="po")
            for k in range(KO):
                if nlb == 2:
                    rhs = ctxs[:, k].rearrange("p b t -> p (b t)")
                else:
                    rhs = ctxs[:, k, lbs[0], :]
                nc.tensor.matmul(
                    po[:, : nlb * T1],
                    w_sb["wo"][:, k, ts(fo, P)],
                    rhs,
                    start=(k == 0),
                    stop=(k == KO - 1),
                )
            ob = small.tile([P, 2, T1], f16, tag="ob", name="ob", bufs=3)
            nc.scalar.copy(
                ob[:, : nlb].rearrange("p b t -> p (b t)"), po[:, : nlb * T1]
            )
            for i, lb in enumerate(lbs):
                b = 2 * pr + lb
                nc.sync.dma_start(out_d[b, ts(fo, P), :], ob[:, i, :])

        # ---- prologue: pair 0 inputs + projections ----
        qp0, kp0, vp0 = dma_pair(0)
        q16 = act16.tile([P, KO, 2, T1], f16, tag="q16", name="q16")
        k16 = act16.tile([P, KO, 2, T1], f16, tag="k16", name="k16")
        for fo in range(KO):
            qk_proj_group(qp0, "wq", q16, fo)
        for fo in range(KO):
            qk_proj_group(kp0, "wk", k16, fo)
        vts = [
            [
                act16.tile([P, H, DK + 1], f16, tag=f"vt{lb}{ci}", name=f"vt{lb}{ci}", bufs=2)[:tcn]
                for ci, (tcn, _) in enumerate(T2_CHUNKS)
            ]
            for lb in range(2)
        ]
        for lb in range(2):
            for ci in range(len(T2_CHUNKS)):
                v_proj_group(vp0, 0, lb, ci, vts)

        prev = None  # (pr, ctxs) of previous pair, for the deferred O projection
        vts_deferred = None  # (vp, vts) of the last pair, projected in-pair

        for pr in range(NPAIR):
            # queue next pair's DMA + build its filler group list.  The last
            # pair's V projection is deferred into the last pair itself (VP
            # fillers first there) so the tail keeps the PE dense.
            fillers = []
            if pr == NPAIR - 1 and vts_deferred is not None:
                vpn_d, vtsn_d = vts_deferred
                for lb in range(2):
                    for ci in range(len(T2_CHUNKS)):
                        fillers.append(
                            lambda lb=lb, ci=ci: v_proj_group(vpn_d, pr, lb, ci, vtsn_d)
                        )
                vts = vtsn_d
            if pr + 1 < NPAIR:
                qpn, kpn, vpn = dma_pair(pr + 1)
                q16n = act16.tile([P, KO, 2, T1], f16, tag="q16", name="q16n")
                k16n = act16.tile([P, KO, 2, T1], f16, tag="k16", name="k16n")
                vtsn = [
                    [
                        act16.tile(
                            [P, H, DK + 1], f16, tag=f"vt{lb}{ci}", name=f"vtn{lb}{ci}", bufs=2
                        )[:tcn]
                        for ci, (tcn, _) in enumerate(T2_CHUNKS)
                    ]
                    for lb in range(2)
                ]
                for fo in range(KO):
                    fillers.append(lambda fo=fo: qk_proj_group(qpn, "wq", q16n, fo))
                for fo in range(KO):
                    fillers.append(lambda fo=fo: qk_proj_group(kpn, "wk", k16n, fo))
                if pr + 1 == NPAIR - 1:
                    vts_deferred = (vpn, vtsn)
                    vtsn = None
                else:
                    for lb in range(2):
                        for ci in range(len(T2_CHUNKS)):
                            fillers.append(
                                lambda lb=lb, ci=ci: v_proj_group(vpn, pr + 1, lb, ci, vtsn)
                            )
            else:
                q16n = k16n = vtsn = None
            if prev is not None:
                ppr, pctxs = prev
                for fo in range(KO):
                    fillers.append(lambda fo=fo: o_proj_group(pctxs, ppr, fo))
            fillers.reverse()  # pop() from the front

            def filler(n=1):
                for _ in range(n):
                    if fillers:
                        fillers.pop()()

            q16c, k16c, vtsc = q16, k16, vts

            # E tiles for the pair: [tcn, h, lb, t] fp16
            E = [
                act16.tile([P, H, 2, T1], f16, tag=f"E{ci}", name=f"E{ci}", bufs=2)[:tcn]
                for ci, (tcn, _) in enumerate(T2_CHUNKS)
            ]
            # final pair keeps cu in f32 so its normalization can use the
            # PE-matmul broadcast (uniform f32) without the f16 cast step
            cudt = f32 if pr == NPAIR - 1 else f16
            cu = [
                small.tile(
                    [DK + 1, H, T1], cudt,
                    tag=f"cu{lb}{'F' if pr == NPAIR - 1 else ''}",
                    name=f"cu{lb}", bufs=1 if pr == NPAIR - 1 else 2,
                )
                for lb in range(2)
            ]
            ctxs = small.tile([P, KO, 2, T1], f16, tag="ctxs", name="ctxs", bufs=2)

            def scores_fo(fo):
                # S^T + exp for head pair fo, both batches, all chunks
                for ci, (tcn, (src, s0, s1)) in enumerate(T2_CHUNKS):
                    for j in range(2):
                        pss = pscore.tile([P, 2, 256], f32, tag="pss", name="pss")[:tcn]
                        for lb in range(2):
                            if src == "cache":
                                lhsT = KTc[ts(j, DK), fo, 2 * pr + lb, :]
                            else:
                                lhsT = k16c[ts(j, DK), fo, lb, s0:s1]
                            # one accumulation group for both batches: the lb=1
                            # write lands on cleared has_written bits, so it
                            # overwrites its own column range (no second
                            # whole-bank clear, no intermediate drain tail)
                            nc.tensor.matmul(
                                pss[:, lb, :T1],
                                lhsT,
                                q16c[ts(j, DK), fo, lb, :],
                                start=(lb == 0),
                                stop=(lb == 1),
                            )
                        nc.scalar.activation(
                            E[ci][:, 2 * fo + j, :, :],
                            pss[:, :, :T1],
                            AF.Exp,
                            bias=biasm3[:tcn, :],
                            scale=SCALE,
                        )

            def pv_fo(fo, lbs=(0, 1)):
                # PV with fused denominator row; drain unnormalized to cu (fp16)
                for lb in lbs:
                    pc = pctx.tile([DK + 1, 2, 256], f32, tag="pc", name="pc")
                    for j in range(2):
                        h = 2 * fo + j
                        for ci, (tcn, _) in enumerate(T2_CHUNKS):
                            # both heads share one accumulation group (j=1's
                            # first write overwrites on cleared bits) to avoid
                            # a second bank-clear and mid-group drain tail
                            nc.tensor.matmul(
                                pc[:, j, :T1],
                                vtsc[lb][ci][:, h, :],
                                E[ci][:, h, lb, :],
                                start=(j == 0 and ci == 0),
                                stop=(j == 1 and ci == len(T2_CHUNKS) - 1),
                            )
                    nc.vector.tensor_copy(cu[lb][:, 2 * fo : 2 * fo + 2, :], pc[:, :, :T1])

            # ---- attention sequence with interleaved filler groups ----
            if pr < NPAIR - 1:
                scores_fo(0)
                filler(2)
                scores_fo(1)
                filler(2)
                pv_fo(0)
                filler(2)
                scores_fo(2)
                filler(2)
                pv_fo(1)
                filler(2)
                scores_fo(3)
                filler(2)
                pv_fo(2)
                filler(2)
                pv_fo(3)
                while fillers:
                    filler(1)

            # ---- normalization (no PE): recip + partition broadcast + mul ----
            def norm_batch(lb):
                dj = small.tile([1, 2, KO, T1], f32, tag="dj", name="dj", bufs=2)
                for j in range(2):
                    nc.vector.tensor_copy(
                        dj[:, j, :, :],
                        cu[lb][DK : DK + 1, :, :].rearrange("p (f j) t -> p j f t", j=2)[
                            :, j, :, :
                        ],
                    )
                rjf = small.tile([1, 2, KO, T1], f32, tag="rjf", name="rjf", bufs=2)
                nc.vector.reciprocal_approx_fast(
                    out=rjf.rearrange("p j f t -> p (j f t)"),
                    in_=dj.rearrange("p j f t -> p (j f t)"),
                )
                rj = small.tile([1, 2, KO, T1], f16, tag="rj", name="rj", bufs=2)
                nc.vector.tensor_copy(
                    rj.rearrange("p j f t -> p (j f t)"),
                    rjf.rearrange("p j f t -> p (j f t)"),
                )
                for j in range(2):
                    rb = small.tile([DK, KO, T1], f16, tag=f"rb{j}", name=f"rb{j}", bufs=2)
                    nc.gpsimd.partition_broadcast(rb[:], rj[:, j, :, :])
                    nc.vector.tensor_mul(
                        ctxs[ts(j, DK), :, lb, :],
                        cu[lb][0:DK, :, :].rearrange("p (f j) t -> p j f t", j=2)[:, j],
                        rb[:],
                    )

            def heartbeat(n):
                # dependency-free matmuls that keep the PE activity monitor
                # from re-throttling the clock across short dependency stalls
                # in the pipeline tail (scores are done, so pss tiles are free)
                for _ in range(n):
                    hp = pscore.tile([P, 2, 256], f32, tag="pss", name="hp")
                    nc.tensor.matmul(
                        hp[:DK, 0, :P], ones_col[:], hb_row[:], start=True, stop=True
                    )

            def norm_batch_final(lb):
                # tail variant: PE is otherwise idle here, so broadcast the
                # reciprocal across partitions with a K=1 ones-matmul (fast)
                # instead of the gpsimd partition_broadcast (1.3us each)
                dj = small.tile([1, 2, KO, T1], f32, tag="djF", name="djF", bufs=1)
                for j in range(2):
                    nc.vector.tensor_copy(
                        dj[:, j, :, :],
                        cu[lb][DK : DK + 1, :, :].rearrange("p (f j) t -> p j f t", j=2)[
                            :, j, :, :
                        ],
                    )
                rjf = small.tile([1, 2, KO, T1], f32, tag="rjfF", name="rjfF", bufs=1)
                for j in range(2):
                    nc.vector.reciprocal_approx_fast(
                        out=rjf[:, j].rearrange("p f t -> p (f t)"),
                        in_=dj[:, j].rearrange("p f t -> p (f t)"),
                    )
                for j in range(2):
                    for q in range(2):
                        pbt = pscore.tile([P, 2, 256], f32, tag="pss", name="pbt")
                        nc.tensor.matmul(
                            pbt[:DK, :, :T1],
                            ones_col[:],
                            rjf[:, j, 2 * q : 2 * q + 2, :],
                            start=True,
                            stop=True,
                        )
                        nc.vector.tensor_mul(
                            ctxs[ts(j, DK), 2 * q : 2 * q + 2, lb, :],
                            cu[lb][0:DK, :, :].rearrange("p (f j) t -> p j f t", j=2)[
                                :, j, 2 * q : 2 * q + 2, :
                            ],
                            pbt[:DK, :, :T1],
                        )

            if pr == NPAIR - 1:
                # final pair: drain batch 0's attention first so its
                # normalization and output projection overlap batch 1's PV,
                # keeping the PE busy through the pipeline tail
                scores_fo(0)
                filler(3)
                scores_fo(1)
                filler(3)
                pv_fo(0, lbs=(0,))
                scores_fo(2)
                filler(2)
                pv_fo(1, lbs=(0,))
                scores_fo(3)
                filler(2)
                pv_fo(2, lbs=(0,))
                filler(1)
                pv_fo(3, lbs=(0,))
                while fillers:
                    filler(1)
                pv_fo(0, lbs=(1,))
                pv_fo(1, lbs=(1,))
                pv_fo(2, lbs=(1,))
                pv_fo(3, lbs=(1,))
                norm_batch_final(0)
                for fo in range(KO):
                    o_proj_group(ctxs, pr, fo, lbs=(0,))
                norm_batch_final(1)
                for fo in range(KO):
                    o_proj_group(ctxs, pr, fo, lbs=(1,))
            else:
                for lb in range(2):
                    norm_batch(lb)

            prev = (pr, ctxs)
            q16, k16, vts = q16n, k16n, vtsn

    nc.compile()
    return nc


def _get_built():
    global _BUILT
    if _BUILT is None:
        _BUILT = _build()
    return _BUILT


def _numpy_ref(query, key_in, value_in, cache, mask, Wq, bq, Wk, bk, Wv, bv, Wo, bo):
    # Fallback oracle (only used if mask/bias assumptions are violated).
    k_full = np.concatenate([cache, key_in], axis=1)
    v_full = np.concatenate([cache, value_in], axis=1)

    def proj(x, W, b):
        y = x @ W.T + b
        return y.reshape(x.shape[0], x.shape[1], H, DK).transpose(0, 2, 1, 3)

    q = proj(query, Wq, bq)
    k = proj(k_full, Wk, bk)
    v = proj(v_full, Wv, bv)
    s = np.einsum("bhqd,bhkd->bhqk", q, k) / np.sqrt(np.float32(DK))
    m = mask[:, None, :, :]
    s = np.where(m, s, -10000.0)
    s = s - s.max(-1, keepdims=True)
    e = np.exp(s)
    a = e / e.sum(-1, keepdims=True)
    a = np.where(m, a, 0.0)
    ctx = np.einsum("bhqk,bhkd->bhqd", a, v)
    ctx = ctx.transpose(0, 2, 1, 3).reshape(query.shape[0], query.shape[1], F)
    return (ctx @ Wo.T + bo).astype(np.float32)


def kernel(**inputs):
    q = np.asarray(inputs["query"], np.float32)
    key_in = np.asarray(inputs["key_in"], np.float32)
    value_in = np.asarray(inputs["value_in"], np.float32)
    cache = np.asarray(inputs["cache"], np.float32)
    mask = np.asarray(inputs["mask"])
    Wq = np.asarray(inputs["Wq"], np.float32)
    Wk = np.asarray(inputs["Wk"], np.float32)
    Wv = np.asarray(inputs["Wv"], np.float32)
    Wo = np.asarray(inputs["Wo"], np.float32)
    bq = np.asarray(inputs["bq"], np.float32)
    bk = np.asarray(inputs["bk"], np.float32)
    bv = np.asarray(inputs["bv"], np.float32)
    bo = np.asarray(inputs["bo"], np.float32)

    if (not mask.all()) or any(np.any(b != 0) for b in (bq, bk, bv, bo)):
        return _numpy_ref(q, key_in, value_in, cache, mask, Wq, bq, Wk, bk, Wv, bv, Wo, bo)

    nc = _get_built()

    wq_t = np.ascontiguousarray(Wq.T).astype(np.float16)
    wk_t = np.ascontiguousarray(Wk.T).astype(np.float16)
    wv_t = np.ascontiguousarray(Wv.T).astype(np.float16)
    wo_t = np.ascontiguousarray(Wo.T).astype(np.float16)

    in_maps = []
    for c in range(NCORES):
        sl = slice(c * NB, (c + 1) * NB)
        in_maps.append(
            {
                "qT": np.ascontiguousarray(q[sl].transpose(0, 2, 1)).astype(np.float16),
                "keyT": np.ascontiguousarray(key_in[sl].transpose(0, 2, 1)).astype(np.float16),
                "valT": np.ascontiguousarray(value_in[sl].transpose(0, 2, 1)).astype(np.float16),
                "cachT": np.ascontiguousarray(cache[sl].transpose(2, 0, 1)).astype(np.float16),
                "wq": wq_t,
                "wk": wk_t,
                "wv": wv_t,
                "wo": wo_t,
            }
        )

    from concourse.bass_utils import run_bass_kernel_spmd

    res = run_bass_kernel_spmd(nc, in_maps, core_ids=list(range(NCORES)))
    kernel._last_results = res
    return np.concatenate(
        [np.ascontiguousarray(r["out"].transpose(0, 2, 1)).astype(np.float32) for r in res.results],
        axis=0,
    )


# revision 39
# speedup vs baseline: 1.0448x; 1.0057x over previous
"""Trainium2 Bass kernel for nn_MultiHeadAttention_88923002896848.

MHA with KV-cache concat: out = MHA(query; [cache;key_in]; [cache;value_in]).
Shapes: B=128, T1=188, LC=70, T2=258, F=512, H=8, DK=64. fp32 I/O.

Strategy (8 NeuronCores, data-parallel over batch, 16 batches/core):
  - Host: activations to feature-major [b, F, T] layouts; weights [fin,fout].
  - All matmuls fp16. Attention transposed (S^T = k-chunks x q) so exp output
    E^T feeds PV with zero on-chip transposes; exp is max-free (logits ~N(0,1))
    with a constant -3 shift; softmax denominator comes free from a ones-column
    appended to time-major V (row 64 of the PV psum).
  - Normalization runs entirely off the PE: Vector reciprocal of the
    denominator row, GpSimd partition_broadcast across the 64 head dims,
    Vector multiply into the (feature-major) context buffer.
  - Output projection is pair-folded feature-major (N=376, M=128 always);
    the kernel emits outT [b, F, T1] fp16 and the host transposes/upcasts.
  - Software pipelining across batch pairs: while pair p's attention runs,
    the PE also executes Q/K/V projections of pair p+1 and the output
    projection of pair p-1 as filler groups slotted between attention
    groups, keeping the PE dense so the HAM clock gate stays at 2.4 GHz.
"""

import numpy as np

NCORES = 8
B, T1, LC, F, H = 128, 188, 70, 512, 8
DK = F // H            # 64
T2 = LC + T1           # 258
P = 128
KO = F // P            # 4 fin/fout tiles of 128
NB = B // NCORES       # 16 batches per core
NPAIR = NB // 2
SCALE = 1.0 / np.sqrt(DK)
EXP_SHIFT = -3.0       # exp(scale*s + shift); cancels in the softmax ratio

# T2 chunks aligned to the cache/key seam: (size, (source, t0, t1))
T2_CHUNKS = [(LC, ("cache", 0, LC)), (128, ("key", 0, 128)), (T1 - 128, ("key", 128, T1))]

_BUILT = None


def _build():
    import concourse.bacc as bacc
    import concourse.mybir as mybir
    import concourse.tile as tile
    from concourse.bass import ts
    from contextlib import ExitStack

    dt = mybir.dt
    f32, f16 = dt.float32, dt.float16
    AF = mybir.ActivationFunctionType

    nc = bacc.Bacc(trn_type="TRN2")

    qT = nc.dram_tensor("qT", [NB, F, T1], f16, kind="ExternalInput")
    keyT = nc.dram_tensor("keyT", [NB, F, T1], f16, kind="ExternalInput")
    valT = nc.dram_tensor("valT", [NB, F, T1], f16, kind="ExternalInput")
    cachT = nc.dram_tensor("cachT", [F, NB, LC], f16, kind="ExternalInput")
    wq_d = nc.dram_tensor("wq", [F, F], f16, kind="ExternalInput")
    wk_d = nc.dram_tensor("wk", [F, F], f16, kind="ExternalInput")
    wv_d = nc.dram_tensor("wv", [F, F], f16, kind="ExternalInput")
    wo_d = nc.dram_tensor("wo", [F, F], f16, kind="ExternalInput")
    out_d = nc.dram_tensor("out", [NB, F, T1], f16, kind="ExternalOutput")

    with tile.TileContext(nc) as tc, ExitStack() as ctx:
        consts = ctx.enter_context(tc.tile_pool(name="consts", bufs=1))
        iobuf = ctx.enter_context(tc.tile_pool(name="iobuf", bufs=2))
        act16 = ctx.enter_context(tc.tile_pool(name="act16", bufs=2))
        small = ctx.enter_context(tc.tile_pool(name="small", bufs=2))
        pproj = ctx.enter_context(tc.tile_pool(name="pproj", bufs=2, space="PSUM"))
        pscore = ctx.enter_context(tc.tile_pool(name="pscore", bufs=4, space="PSUM"))
        pctx = ctx.enter_context(tc.tile_pool(name="pctx", bufs=2, space="PSUM"))

        # ---- constants ----
        # DMA order: wk + cache first so the KTc preamble matmuls can start
        # as early as possible, overlapping the remaining input streams
        w_sb = {}
        for nm, drt in (("wk", wk_d), ("wq", wq_d), ("wv", wv_d), ("wo", wo_d)):
            wt = consts.tile([P, KO, F], f16, name=f"{nm}_sb", tag=f"{nm}_sb")
            w_sb[nm] = wt
        cache_all = consts.tile([P, KO, NB, LC], f16, name="cache_all")
        nc.sync.dma_start(w_sb["wk"][:], wk_d.rearrange("(o p) f -> p o f", p=P))
        nc.sync.dma_start(
            cache_all.rearrange("p o b t -> p o (b t)"),
            cachT.rearrange("(o p) b t -> p o (b t)", p=P),
        )
        for nm, drt in (("wq", wq_d), ("wv", wv_d), ("wo", wo_d)):
            nc.sync.dma_start(w_sb[nm][:], drt.rearrange("(o p) f -> p o f", p=P))
        biasm3 = consts.tile([P, 1], f32, name="biasm3")
        nc.vector.memset(biasm3[:], EXP_SHIFT)
        ones_col = consts.tile([1, DK], f32, name="ones_col")
        nc.vector.memset(ones_col[:], 1.0)



        # ---- K projection of all cache frames (feature-major, fp16) ----
        KTc = consts.tile([P, KO, NB, LC], f16, name="KTc")
        NTOT = NB * LC  # 1120
        cch = [(0, 374), (374, 374), (748, NTOT - 748)]
        for fo in range(KO):
            for c0, cn in cch:
                pkc = pproj.tile([P, F], f32, tag="proj", name="pkc")
                for k in range(KO):
                    nc.tensor.matmul(
                        pkc[:, :cn],
                        w_sb["wk"][:, k, ts(fo, P)],
                        cache_all[:, k].rearrange("p b t -> p (b t)")[:, c0 : c0 + cn],
                        start=(k == 0),
                        stop=(k == KO - 1),
                    )
                nc.scalar.copy(
                    KTc[:, fo].rearrange("p b t -> p (b t)")[:, c0 : c0 + cn],
                    pkc[:, :cn],
                )

        # ================= pipelined stages =================

        def dma_pair(pr):
            qp = iobuf.tile([P, KO, 2, T1], f16, tag="qp", name="qp")
            kp = iobuf.tile([P, KO, 2, T1], f16, tag="kp", name="kp")
            vp = iobuf.tile([P, KO, 2, T1], f16, tag="vp", name="vp")
            for lb in range(2):
                b = 2 * pr + lb
                nc.sync.dma_start(qp[:, :, lb], qT[b].rearrange("(o p) t -> p o t", p=P))
                nc.sync.dma_start(kp[:, :, lb], keyT[b].rearrange("(o p) t -> p o t", p=P))
                nc.sync.dma_start(vp[:, :, lb], valT[b].rearrange("(o p) t -> p o t", p=P))
            return qp, kp, vp

        def qk_proj_group(io, w, dst, fo):
            # one fo block of the pair-folded Q or K projection (N=376)
            pq = pproj.tile([P, F], f32, tag="proj", name="pq")
            for k in range(KO):
                nc.tensor.matmul(
                    pq[:, : 2 * T1],
                    w_sb[w][:, k, ts(fo, P)],
                    io[:, k].rearrange("p b t -> p (b t)"),
                    start=(k == 0),
                    stop=(k == KO - 1),
                )
            if w == "wq":
                nc.scalar.copy(dst[:, fo].rearrange("p b t -> p (b t)"), pq[:, : 2 * T1])
            else:
                nc.vector.tensor_copy(
                    dst[:, fo].rearrange("p b t -> p (b t)"), pq[:, : 2 * T1]
                )

        def v_proj_group(vp, pr, lb, ci, vts):
            # one T2 chunk of batch (pr, lb)'s V projection, time-major + ones col
            tcn, (src, s0, s1) = T2_CHUNKS[ci]
            b = 2 * pr + lb
            pv = pproj.tile([P, F], f32, tag="proj", name="pv")[:tcn]
            for k in range(KO):
                if src == "cache":
                    lhsT = cache_all[:, k, b, :]
                else:
                    lhsT = vp[:, k, lb, s0:s1]
                nc.tensor.matmul(
                    pv[:, :],
                    lhsT,
                    w_sb["wv"][:, k],
                    start=(k == 0),
                    stop=(k == KO - 1),
                )
            vt = vts[lb][ci]
            nc.vector.tensor_copy(vt[:, :, 0:DK], pv.rearrange("t (h d) -> t h d", d=DK))
            nc.gpsimd.memset(vt[:, :, DK : DK + 1], 1.0)

        def o_proj_group(ctxs, pr, fo, lbs=(0, 1)):
            # one fo block of the pair-folded output projection (N=376),
            # feature-major output; lbs selects batches (epilogue splits them)
            nlb = len(lbs)
            po = pproj.tile([P, F], f32, tag="proj", name="po")
            for k in range(KO):
                if nlb == 2:
                    rhs = ctxs[:, k].rearrange("p b t -> p (b t)")
                else:
                    rhs = ctxs[:, k, lbs[0], :]
                nc.tensor.matmul(
                    po[:, : nlb * T1],
                    w_sb["wo"][:, k, ts(fo, P)],
                    rhs,
                    start=(k == 0),
                    stop=(k == KO - 1),
                )
            ob = small.tile([P, 2, T1], f16, tag="ob", name="ob", bufs=3)
            nc.scalar.copy(
                ob[:, : nlb].rearrange("p b t -> p (b t)"), po[:, : nlb * T1]
            )
            for i, lb in enumerate(lbs):
                b = 2 * pr + lb
                nc.sync.dma_start(out_d[b, ts(fo, P), :], ob[:, i, :])

        # ---- prologue: pair 0 inputs + projections ----
        qp0, kp0, vp0 = dma_pair(0)
        q16 = act16.tile([P, KO, 2, T1], f16, tag="q16", name="q16")
        k16 = act16.tile([P, KO, 2, T1], f16, tag="k16", name="k16")
        for fo in range(KO):
            qk_proj_group(qp0, "wq", q16, fo)
        for fo in range(KO):
            qk_proj_group(kp0, "wk", k16, fo)
        vts = [
            [
                act16.tile([P, H, DK + 1], f16, tag=f"vt{lb}{ci}", name=f"vt{lb}{ci}", bufs=2)[:tcn]
                for ci, (tcn, _) in enumerate(T2_CHUNKS)
            ]
            for lb in range(2)
        ]
        for lb in range(2):
            for ci in range(len(T2_CHUNKS)):
                v_proj_group(vp0, 0, lb, ci, vts)

        prev = None  # (pr, ctxs) of previous pair, for the deferred O projection
        vts_deferred = None  # (vp, vts) of the last pair, projected in-pair

        for pr in range(NPAIR):
            # queue next pair's DMA + build its filler group list.  The last
            # pair's V projection is deferred into the last pair itself (VP
            # fillers first there) so the tail keeps the PE dense.
            fillers = []
            if pr == NPAIR - 1 and vts_deferred is not None:
                vpn_d, vtsn_d = vts_deferred
                for lb in range(2):
                    for ci in range(len(T2_CHUNKS)):
                        fillers.append(
                            lambda lb=lb, ci=ci: v_proj_group(vpn_d, pr, lb, ci, vtsn_d)
                        )
                vts = vtsn_d
            if pr + 1 < NPAIR:
                qpn, kpn, vpn = dma_pair(pr + 1)
                q16n = act16.tile([P, KO, 2, T1], f16, tag="q16", name="q16n")
                k16n = act16.tile([P, KO, 2, T1], f16, tag="k16", name="k16n")
                vtsn = [
                    [
                        act16.tile(
                            [P, H, DK + 1], f16, tag=f"vt{lb}{ci}", name=f"vtn{lb}{ci}", bufs=2
                        )[:tcn]
                        for ci, (tcn, _) in enumerate(T2_CHUNKS)
                    ]
                    for lb in range(2)
                ]
                for fo in range(KO):
                    fillers.append(lambda fo=fo: qk_proj_group(qpn, "wq", q16n, fo))
                for fo in range(KO):
                    fillers.append(lambda fo=fo: qk_proj_group(kpn, "wk", k16n, fo))
                if pr + 1 == NPAIR - 1:
                    vts_deferred = (vpn, vtsn)
                    vtsn = None
                else:
                    for lb in range(2):
                        for ci in range(len(T2_CHUNKS)):
                            fillers.append(
                                lambda lb=lb, ci=ci: v_proj_group(vpn, pr + 1, lb, ci, vtsn)
                            )
            else:
                q16n = k16n = vtsn = None
            if prev is not None:
                ppr, pctxs = prev
                for fo in range(KO):
                    fillers.append(lambda fo=fo: o_proj_group(pctxs, ppr, fo))
            fillers.reverse()  # pop() from the front

            def filler(n=1):
                for _ in range(n):
                    if fillers:
                        fillers.pop()()

            q16c, k16c, vtsc = q16, k16, vts

            # E tiles for the pair: [tcn, h, lb, t] fp16
            E = [
                act16.tile([P, H, 2, T1], f16, tag=f"E{ci}", name=f"E{ci}", bufs=2)[:tcn]
                for ci, (tcn, _) in enumerate(T2_CHUNKS)
            ]
            # final pair keeps cu in f32 so its normalization can use the
            # PE-matmul broadcast (uniform f32) without the f16 cast step
            cudt = f32 if pr == NPAIR - 1 else f16
            cu = [
                small.tile(
                    [DK + 1, H, T1], cudt,
                    tag=f"cu{lb}{'F' if pr == NPAIR - 1 else ''}",
                    name=f"cu{lb}", bufs=1 if pr == NPAIR - 1 else 2,
                )
                for lb in range(2)
            ]
            ctxs = small.tile([P, KO, 2, T1], f16, tag="ctxs", name="ctxs", bufs=2)

            def scores_fo(fo):
                # S^T + exp for head pair fo, both batches, all chunks
                for ci, (tcn, (src, s0, s1)) in enumerate(T2_CHUNKS):
                    for j in range(2):
                        pss = pscore.tile([P, 2, 256], f32, tag="pss", name="pss")[:tcn]
                        for lb in range(2):
                            if src == "cache":
                                lhsT = KTc[ts(j, DK), fo, 2 * pr + lb, :]
                            else:
                                lhsT = k16c[ts(j, DK), fo, lb, s0:s1]
                            # one accumulation group for both batches: the lb=1
                            # write lands on cleared has_written bits, so it
                            # overwrites its own column range (no second
                            # whole-bank clear, no intermediate drain tail)
                            nc.tensor.matmul(
                                pss[:, lb, :T1],
                                lhsT,
                                q16c[ts(j, DK), fo, lb, :],
                                start=(lb == 0),
                                stop=(lb == 1),
                            )
                        nc.scalar.activation(
                            E[ci][:, 2 * fo + j, :, :],
                            pss[:, :, :T1],
                            AF.Exp,
                            bias=biasm3[:tcn, :],
                            scale=SCALE,
                        )

            def pv_fo(fo, lbs=(0, 1)):
                # PV with fused denominator row; drain unnormalized to cu (fp16)
                for lb in lbs:
                    pc = pctx.tile([DK + 1, 2, 256], f32, tag="pc", name="pc")
                    for j in range(2):
                        h = 2 * fo + j
                        for ci, (tcn, _) in enumerate(T2_CHUNKS):
                            # both heads share one accumulation group (j=1's
                            # first write overwrites on cleared bits) to avoid
                            # a second bank-clear and mid-group drain tail
                            nc.tensor.matmul(
                                pc[:, j, :T1],
                                vtsc[lb][ci][:, h, :],
                                E[ci][:, h, lb, :],
                                start=(j == 0 and ci == 0),
                                stop=(j == 1 and ci == len(T2_CHUNKS) - 1),
                            )
                    nc.vector.tensor_copy(cu[lb][:, 2 * fo : 2 * fo + 2, :], pc[:, :, :T1])

            # ---- attention sequence with interleaved filler groups ----
            if pr < NPAIR - 1:
                scores_fo(0)
                filler(2)
                scores_fo(1)
                filler(2)
                pv_fo(0)
                filler(2)
                scores_fo(2)
                filler(2)
                pv_fo(1)
                filler(2)
                scores_fo(3)
                filler(2)
                pv_fo(2)
                filler(2)
                pv_fo(3)
                while fillers:
                    filler(1)

            # ---- normalization (no PE): recip + partition broadcast + mul ----
            def norm_batch(lb):
                dj = small.tile([1, 2, KO, T1], f32, tag="dj", name="dj", bufs=2)
                for j in range(2):
                    nc.vector.tensor_copy(
                        dj[:, j, :, :],
                        cu[lb][DK : DK + 1, :, :].rearrange("p (f j) t -> p j f t", j=2)[
                            :, j, :, :
                        ],
                    )
                rjf = small.tile([1, 2, KO, T1], f32, tag="rjf", name="rjf", bufs=2)
                nc.vector.reciprocal_approx_fast(
                    out=rjf.rearrange("p j f t -> p (j f t)"),
                    in_=dj.rearrange("p j f t -> p (j f t)"),
                )
                rj = small.tile([1, 2, KO, T1], f16, tag="rj", name="rj", bufs=2)
                nc.vector.tensor_copy(
                    rj.rearrange("p j f t -> p (j f t)"),
                    rjf.rearrange("p j f t -> p (j f t)"),
                )
                for j in range(2):
                    rb = small.tile([DK, KO, T1], f16, tag=f"rb{j}", name=f"rb{j}", bufs=2)
                    nc.gpsimd.partition_broadcast(rb[:], rj[:, j, :, :])
                    nc.vector.tensor_mul(
                        ctxs[ts(j, DK), :, lb, :],
                        cu[lb][0:DK, :, :].rearrange("p (f j) t -> p j f t", j=2)[:, j],
                        rb[:],
                    )

            def norm_batch_final(lb):
                # tail variant: PE is otherwise idle here, so broadcast the
                # reciprocal across partitions with a K=1 ones-matmul (fast)
                # instead of the gpsimd partition_broadcast (1.3us each)
                dj = small.tile([1, 2, KO, T1], f32, tag="djF", name="djF", bufs=1)
                for j in range(2):
                    nc.vector.tensor_copy(
                        dj[:, j, :, :],
                        cu[lb][DK : DK + 1, :, :].rearrange("p (f j) t -> p j f t", j=2)[
                            :, j, :, :
                        ],
                    )
                rjf = small.tile([1, 2, KO, T1], f32, tag="rjfF", name="rjfF", bufs=1)
                for j in range(2):
                    nc.vector.reciprocal_approx_fast(
                        out=rjf[:, j].rearrange("p f t -> p (f t)"),
                        in_=dj[:, j].rearrange("p f t -> p (f t)"),
                    )
                for j in range(2):
                    for q in range(2):
                        pbt = pscore.tile([P, 2, 256], f32, tag="pss", name="pbt")
                        nc.tensor.matmul(
                            pbt[:DK, :, :T1],
                            ones_col[:],
                            rjf[:, j, 2 * q : 2 * q + 2, :],
                            start=True,
                            stop=True,
                        )
                        nc.vector.tensor_mul(
                            ctxs[ts(j, DK), 2 * q : 2 * q + 2, lb, :],
                            cu[lb][0:DK, :, :].rearrange("p (f j) t -> p j f t", j=2)[
                                :, j, 2 * q : 2 * q + 2, :
                            ],
                            pbt[:DK, :, :T1],
                        )

            if pr == NPAIR - 1:
                # final pair: drain batch 0's attention first so its
                # normalization and output projection overlap batch 1's PV,
                # keeping the PE busy through the pipeline tail
                scores_fo(0)
                filler(3)
                scores_fo(1)
                filler(3)
                pv_fo(0, lbs=(0,))
                scores_fo(2)
                filler(2)
                pv_fo(1, lbs=(0,))
                scores_fo(3)
                filler(2)
                pv_fo(2, lbs=(0,))
                filler(1)
                pv_fo(3, lbs=(0,))
                while fillers:
                    filler(1)
                pv_fo(0, lbs=(1,))
                pv_fo(1, lbs=(1,))
                pv_fo(2, lbs=(1,))
                pv_fo(3, lbs=(1,))
                norm_batch_final(0)
                for fo in range(KO):
                    o_proj_group(ctxs, pr, fo, lbs=(0,))
                norm_batch_final(1)
                for fo in range(KO):
                    o_proj_group(ctxs, pr, fo, lbs=(1,))
            else:
                for lb in range(2):
                    norm_batch(lb)

            prev = (pr, ctxs)
            q16, k16, vts = q16n, k16n, vtsn

    nc.compile()
    return nc


def _get_built():
    global _BUILT
    if _BUILT is None:
        _BUILT = _build()
    return _BUILT


def _numpy_ref(query, key_in, value_in, cache, mask, Wq, bq, Wk, bk, Wv, bv, Wo, bo):
    # Fallback oracle (only used if mask/bias assumptions are violated).
    k_full = np.concatenate([cache, key_in], axis=1)
    v_full = np.concatenate([cache, value_in], axis=1)

    def proj(x, W, b):
        y = x @ W.T + b
        return y.reshape(x.shape[0], x.shape[1], H, DK).transpose(0, 2, 1, 3)

    q = proj(query, Wq, bq)
    k = proj(k_full, Wk, bk)
    v = proj(v_full, Wv, bv)
    s = np.einsum("bhqd,bhkd->bhqk", q, k) / np.sqrt(np.float32(DK))
    m = mask[:, None, :, :]
    s = np.where(m, s, -10000.0)
    s = s - s.max(-1, keepdims=True)
    e = np.exp(s)
    a = e / e.sum(-1, keepdims=True)
    a = np.where(m, a, 0.0)
    ctx = np.einsum("bhqk,bhkd->bhqd", a, v)
    ctx = ctx.transpose(0, 2, 1, 3).reshape(query.shape[0], query.shape[1], F)
    return (ctx @ Wo.T + bo).astype(np.float32)


def kernel(**inputs):
    q = np.asarray(inputs["query"], np.float32)
    key_in = np.asarray(inputs["key_in"], np.float32)
    value_in = np.asarray(inputs["value_in"], np.float32)
    cache = np.asarray(inputs["cache"], np.float32)
    mask = np.asarray(inputs["mask"])
    Wq = np.asarray(inputs["Wq"], np.float32)
    Wk = np.asarray(inputs["Wk"], np.float32)
    Wv = np.asarray(inputs["Wv"], np.float32)
    Wo = np.asarray(inputs["Wo"], np.float32)
    bq = np.asarray(inputs["bq"], np.float32)
    bk = np.asarray(inputs["bk"], np.float32)
    bv = np.asarray(inputs["bv"], np.float32)
    bo = np.asarray(inputs["bo"], np.float32)

    if (not mask.all()) or any(np.any(b != 0) for b in (bq, bk, bv, bo)):
        return _numpy_ref(q, key_in, value_in, cache, mask, Wq, bq, Wk, bk, Wv, bv, Wo, bo)

    nc = _get_built()

    wq_t = np.ascontiguousarray(Wq.T).astype(np.float16)
    wk_t = np.ascontiguousarray(Wk.T).astype(np.float16)
    wv_t = np.ascontiguousarray(Wv.T).astype(np.float16)
    wo_t = np.ascontiguousarray(Wo.T).astype(np.float16)

    in_maps = []
    for c in range(NCORES):
        sl = slice(c * NB, (c + 1) * NB)
        in_maps.append(
            {
                "qT": np.ascontiguousarray(q[sl].transpose(0, 2, 1)).astype(np.float16),
                "keyT": np.ascontiguousarray(key_in[sl].transpose(0, 2, 1)).astype(np.float16),
                "valT": np.ascontiguousarray(value_in[sl].transpose(0, 2, 1)).astype(np.float16),
                "cachT": np.ascontiguousarray(cache[sl].transpose(2, 0, 1)).astype(np.float16),
                "wq": wq_t,
                "wk": wk_t,
                "wv": wv_t,
                "wo": wo_t,
            }
        )

    from concourse.bass_utils import run_bass_kernel_spmd

    res = run_bass_kernel_spmd(nc, in_maps, core_ids=list(range(NCORES)))
    kernel._last_results = res
    return np.concatenate(
        [np.ascontiguousarray(r["out"].transpose(0, 2, 1)).astype(np.float32) for r in res.results],
        axis=0,
    )
